# revision 9
# baseline (speedup 1.0000x reference)
"""Fused transformer block (LN1 -> MHA -> LN2 -> MLP, layerscale residuals)
for 8 Trainium2 NeuronCores.

Sharding: core c in 0..7 owns batch b = c//4 and query-row quarter r = c%4
(512 rows of 2048).  Each core recomputes K/V for its whole batch (no
cross-core communication), computes attention + proj + MLP for its own 512
rows, and writes a [512, 768] fp32 output slice.  Host assembles the full
[2, 2048, 768] output.

Dataflow is kept transposed where it kills transposes:
  LN1^T [C, rows] -> Q^T/K^T [cols, rows] (via w_qkv as lhsT) and V row-major
  [rows, cols] (via LN1^T as lhsT).  Scores are computed as S^T [keys, q]
  (lhsT = K^T head slice, rhs = Q^T head slice), exp'd on ACT with the 1/8
  scale, masked multiplicatively (exact: masked -> 0), and fed straight into
  AV as the moving operand with a ones-augmented V as lhsT so the softmax
  denominator falls out of the matmul's 65th output partition.  o^T [d, q]
  then feeds proj as lhsT; fc1 produces g^T [4C, q] which feeds fc2 as lhsT.

Host-side folding: LN gains/biases are absorbed into w_qkv/w1 + biases,
layerscales into w_proj/w2 + biases, so the kernel only ever normalizes
(x - mu) * rsigma.
"""

from contextlib import ExitStack

import numpy as np
import ml_dtypes

import concourse.bacc as bacc
import concourse.bass as bass
import concourse.tile as tile
from concourse import mybir
from concourse.bass_utils import run_bass_kernel_spmd
from concourse.masks import make_identity

BF16 = ml_dtypes.bfloat16

B, N, C, H, D = 2, 2048, 768, 12, 64
F = 4 * C            # 3072
P = 128
NCORES = 8
GPB = NCORES // B    # cores (row-quarters) per batch = 4
RPC = N // GPB       # rows per core = 512
KT = N // P          # key tiles = 16
CT = C // P          # channel tiles = 6
FT = F // P          # mlp hidden tiles = 24
QT = RPC // P        # own-row tiles = 4
EPS = 1e-5

f32 = mybir.dt.float32
bf16 = mybir.dt.bfloat16


def _ln_stats(nc, pool, xt, eps_tile, rows=P):
    """Return (mean, rstd) [rows,1] f32 for xt [rows, C] f32."""
    xr = xt.rearrange("p (s g) -> p s g", g=256)
    stats = pool.tile([P, 3, nc.vector.BN_STATS_DIM], f32, tag="ln_stats")
    for s in range(3):
        nc.vector.bn_stats(out=stats[:rows, s, :], in_=xr[:rows, s, :])
    mv = pool.tile([P, nc.vector.BN_AGGR_DIM], f32, tag="ln_mv")
    nc.vector.bn_aggr(out=mv[:rows], in_=stats[:rows])
    mean = mv[:rows, 0:1]
    rstd = pool.tile([P, 1], f32, tag="ln_rstd")
    # rstd <- 1/sqrt(var + eps)
    nc.scalar.activation(
        out=rstd[:rows], in_=mv[:rows, 1:2],
        func=mybir.ActivationFunctionType.Sqrt,
        bias=eps_tile[:rows], scale=1.0,
    )
    nc.vector.reciprocal(out=rstd[:rows], in_=rstd[:rows])
    return mean, rstd


def build_nc():
    nc = bacc.Bacc(None, target_bir_lowering=False)

    xb = nc.dram_tensor("xb", [N, C], f32, kind="ExternalInput")
    maskt = nc.dram_tensor("maskt", [N, RPC], bf16, kind="ExternalInput")
    wqkv = nc.dram_tensor("wqkv", [C, 3 * C], bf16, kind="ExternalInput")
    bqkv = nc.dram_tensor("bqkv", [3 * C], f32, kind="ExternalInput")
    wproj = nc.dram_tensor("wproj", [D, H, C], bf16, kind="ExternalInput")
    bproj = nc.dram_tensor("bproj", [C], f32, kind="ExternalInput")
    w1d = nc.dram_tensor("w1d", [C, F], bf16, kind="ExternalInput")
    b1d = nc.dram_tensor("b1d", [F], f32, kind="ExternalInput")
    w2d = nc.dram_tensor("w2d", [F, C], bf16, kind="ExternalInput")
    b2d = nc.dram_tensor("b2d", [C], f32, kind="ExternalInput")
    outd = nc.dram_tensor("out", [RPC, C], f32, kind="ExternalOutput")

    def bcast_ap(vec, n):
        # [n] dram vector -> [P, n] partition-broadcast AP
        if not isinstance(vec, bass.AP):
            vec = vec[:]
        return bass.AP(tensor=vec.tensor, offset=vec.offset,
                       ap=[[0, P], *vec.ap])

    with tile.TileContext(nc) as tc, ExitStack() as root:
        consts = root.enter_context(tc.tile_pool(name="consts", bufs=1))
        ident = consts.tile([P, P], bf16)
        make_identity(nc, ident)
        ones_sb = consts.tile([P, D], f32)
        nc.vector.memset(ones_sb, 1.0)
        eps_tile = consts.tile([P, 1], f32)
        nc.vector.memset(eps_tile, EPS)
        bqkv_sb = consts.tile([P, 18], f32)
        nc.sync.dma_start(out=bqkv_sb, in_=bqkv.rearrange("(t p) -> p t", p=P))
        b1_sb = consts.tile([P, FT], f32)
        nc.sync.dma_start(out=b1_sb, in_=b1d.rearrange("(t p) -> p t", p=P))
        bproj_bc = consts.tile([P, C], f32)
        nc.gpsimd.dma_start(out=bproj_bc, in_=bcast_ap(bproj, C))
        b2_bc = consts.tile([P, C], f32)
        nc.gpsimd.dma_start(out=b2_bc, in_=bcast_ap(b2d, C))
        bv_bc = consts.tile([P, C], f32)
        nc.gpsimd.dma_start(out=bv_bc, in_=bcast_ap(bqkv[2 * C:3 * C], C))

        # Long-lived pools, created in reverse order of release (LIFO alloc):
        x1p = root.enter_context(tc.tile_pool(name="x1_p", bufs=1))
        ln2p = root.enter_context(tc.tile_pool(name="ln2t_p", bufs=1))
        x1_sb = x1p.tile([P, QT, C], f32)
        ln2t = ln2p.tile([P, CT, RPC], bf16)
        sBC = root.enter_context(ExitStack())
        otp = sBC.enter_context(tc.tile_pool(name="ot_p", bufs=1))
        ot_sb = otp.tile([D, H, RPC], bf16)

        # ---------------- Phase A: LN1 + Q^T/K^T/V ----------------
        # Pools living through attention (freed before MLP):
        sAB = root.enter_context(ExitStack())
        qk_pool = sAB.enter_context(tc.tile_pool(name="qk_sb", bufs=1))
        vaug_pool = sAB.enter_context(tc.tile_pool(name="vaug_sb", bufs=1))
        qt_sb = qk_pool.tile([P, CT, RPC], bf16)
        kt_sb = qk_pool.tile([P, CT, N], bf16)
        maskt_sb = qk_pool.tile([P, KT, RPC], bf16)
        nc.sync.dma_start(out=maskt_sb,
                          in_=maskt.rearrange("(t p) q -> p t q", p=P))
        vaug = vaug_pool.tile([P, KT, H, D + 1], bf16)
        nc.vector.memset(vaug[:, :, :, D:D + 1], 1.0)

        with ExitStack() as sA:
            wp = sA.enter_context(tc.tile_pool(name="wqkv_p", bufs=1))
            lp = sA.enter_context(tc.tile_pool(name="ln1t_p", bufs=1))
            wqkv_sb = wp.tile([P, CT, 3 * C], bf16)
            nc.sync.dma_start(out=wqkv_sb,
                              in_=wqkv.rearrange("(t p) o -> p t o", p=P))
            ln1t = lp.tile([P, CT, N], bf16)

            lnw = sA.enter_context(tc.tile_pool(name="ln1_work", bufs=3))
            ps_tp = sA.enter_context(tc.tile_pool(name="ps_tp", bufs=2, space="PSUM"))
            for rt in range(KT):
                xt = lnw.tile([P, C], f32, tag="xt")
                nc.sync.dma_start(out=xt, in_=xb[rt * P:(rt + 1) * P, :])
                mean, rstd = _ln_stats(nc, lnw, xt, eps_tile)
                tbf = lnw.tile([P, C], bf16, tag="tbf")
                nc.vector.tensor_scalar(
                    out=tbf, in0=xt, scalar1=mean, scalar2=rstd,
                    op0=mybir.AluOpType.subtract, op1=mybir.AluOpType.mult)
                for ct in range(CT):
                    tp = ps_tp.tile([P, P], bf16, tag="tp")
                    nc.tensor.transpose(tp, tbf[:, ct * P:(ct + 1) * P], ident)
                    nc.vector.tensor_copy(
                        out=ln1t[:, ct, rt * P:(rt + 1) * P], in_=tp)

            # Q^T (own rows), K^T (all rows), V (all rows)
            ps_qk = sA.enter_context(tc.tile_pool(name="ps_qk", bufs=2, space="PSUM"))
            ps_v = sA.enter_context(tc.tile_pool(name="ps_v", bufs=2, space="PSUM"))
            for mt in range(CT):
                ps = ps_qk.tile([P, RPC], f32, tag="psqk")
                for kt in range(CT):
                    nc.tensor.matmul(
                        ps, lhsT=wqkv_sb[:, kt, mt * P:(mt + 1) * P],
                        rhs=ln1t[:, kt, 0:RPC],
                        start=(kt == 0), stop=(kt == CT - 1))
                nc.vector.tensor_scalar(
                    out=qt_sb[:, mt, :], in0=ps,
                    scalar1=bqkv_sb[:, mt:mt + 1], scalar2=None,
                    op0=mybir.AluOpType.add)
            for mt in range(CT):
                for nt in range(N // 512):
                    ps = ps_qk.tile([P, 512], f32, tag="psqk")
                    for kt in range(CT):
                        nc.tensor.matmul(
                            ps,
                            lhsT=wqkv_sb[:, kt, (CT + mt) * P:(CT + mt + 1) * P],
                            rhs=ln1t[:, kt, nt * 512:(nt + 1) * 512],
                            start=(kt == 0), stop=(kt == CT - 1))
                    nc.vector.tensor_scalar(
                        out=kt_sb[:, mt, nt * 512:(nt + 1) * 512],
                        in0=ps, scalar1=bqkv_sb[:, CT + mt:CT + mt + 1],
                        scalar2=None, op0=mybir.AluOpType.add)
            for mt in range(KT):
                ps = ps_v.tile([P, C], f32, tag="psv")
                for c0, c1 in ((0, 512), (512, 768)):
                    for kt in range(CT):
                        nc.tensor.matmul(
                            ps[:, c0:c1],
                            lhsT=ln1t[:, kt, mt * P:(mt + 1) * P],
                            rhs=wqkv_sb[:, kt, 2 * C + c0:2 * C + c1],
                            start=(kt == 0), stop=(kt == CT - 1))
                nc.vector.tensor_tensor(
                    out=vaug[:, mt, :, 0:D],
                    in0=ps.rearrange("p (h d) -> p h d", d=D),
                    in1=bv_bc.rearrange("p (h d) -> p h d", d=D),
                    op=mybir.AluOpType.add)

        # ---------------- Phase B: attention ----------------
        with ExitStack() as sB:
            ptp = sB.enter_context(tc.tile_pool(name="pt_p", bufs=2))
            recp = sB.enter_context(tc.tile_pool(name="rec_p", bufs=2))
            ps_s = sB.enter_context(tc.tile_pool(name="ps_s", bufs=2, space="PSUM"))
            ps_av = sB.enter_context(tc.tile_pool(name="ps_av", bufs=2, space="PSUM"))
            ps_bc = sB.enter_context(tc.tile_pool(name="ps_bc", bufs=2, space="PSUM"))
            for h in range(H):
                po = D * (h % 2)
                mt = h // 2
                pt = ptp.tile([P, KT, RPC], bf16, tag="pt")
                for g in range(KT // 2):
                    sps = ps_s.tile([P, 2, RPC], f32, tag="sps")
                    for j in range(2):
                        k0 = (g * 2 + j) * P
                        nc.tensor.matmul(
                            sps[:, j, :],
                            lhsT=kt_sb[po:po + D, mt, k0:k0 + P],
                            rhs=qt_sb[po:po + D, mt, :],
                            start=True, stop=True)
                    nc.scalar.activation(
                        out=pt[:, g * 2:g * 2 + 2, :], in_=sps,
                        func=mybir.ActivationFunctionType.Exp,
                        scale=float(D) ** -0.5)
                    nc.vector.tensor_tensor(
                        out=pt[:, g * 2:g * 2 + 2, :],
                        in0=pt[:, g * 2:g * 2 + 2, :],
                        in1=maskt_sb[:, g * 2:g * 2 + 2, :],
                        op=mybir.AluOpType.mult)
                avps = ps_av.tile([D + 1, RPC], f32, tag="avps")
                for kt in range(KT):
                    nc.tensor.matmul(
                        avps, lhsT=vaug[:, kt, h, :], rhs=pt[:, kt, :],
                        start=(kt == 0), stop=(kt == KT - 1))
                rec = recp.tile([D + 1, RPC], f32, tag="rec")
                nc.vector.reciprocal(out=rec[D:D + 1, :], in_=avps[D:D + 1, :])
                bcps = ps_bc.tile([D, RPC], f32, tag="bcps")
                nc.tensor.matmul(bcps, lhsT=ones_sb[D:D + 1, 0:D],
                                 rhs=rec[D:D + 1, :], start=True, stop=True)
                rbc = recp.tile([D, RPC], f32, tag="rbc")
                nc.vector.tensor_copy(out=rbc, in_=bcps)
                nc.vector.tensor_tensor(
                    out=ot_sb[:, h, :], in0=avps[0:D, :], in1=rbc,
                    op=mybir.AluOpType.mult)
        sAB.close()  # free qt/kt/vaug before the MLP weights arrive

        # ---------------- Phase C: proj + residual + LN2 ----------------
        with ExitStack() as sC:
            wpp = sC.enter_context(tc.tile_pool(name="wproj_p", bufs=1))
            xqp = sC.enter_context(tc.tile_pool(name="xq_p", bufs=2))
            ln2w = sC.enter_context(tc.tile_pool(name="ln2_work", bufs=3))
            ps_pj = sC.enter_context(tc.tile_pool(name="ps_pj", bufs=2, space="PSUM"))
            ps_t2 = sC.enter_context(tc.tile_pool(name="ps_t2", bufs=2, space="PSUM"))
            wproj_sb = wpp.tile([D, H, C], bf16)
            nc.sync.dma_start(out=wproj_sb, in_=wproj[:, :, :])
            for qt in range(QT):
                ps = ps_pj.tile([P, C], f32, tag="pspj")
                q0 = qt * P
                for c0, c1 in ((0, 512), (512, 768)):
                    for h in range(H):
                        nc.tensor.matmul(
                            ps[:, c0:c1], lhsT=ot_sb[:, h, q0:q0 + P],
                            rhs=wproj_sb[:, h, c0:c1],
                            start=(h == 0), stop=(h == H - 1))
                xq = xqp.tile([P, C], f32, tag="xq")
                nc.sync.dma_start(out=xq, in_=xb[q0:q0 + P, :])
                nc.vector.tensor_tensor(
                    out=x1_sb[:, qt, :], in0=ps, in1=xq, op=mybir.AluOpType.add)
                nc.vector.tensor_tensor(
                    out=x1_sb[:, qt, :], in0=x1_sb[:, qt, :], in1=bproj_bc,
                    op=mybir.AluOpType.add)
                mean2, rstd2 = _ln_stats(nc, ln2w, x1_sb[:, qt, :], eps_tile)
                t2 = ln2w.tile([P, C], bf16, tag="t2")
                nc.vector.tensor_scalar(
                    out=t2, in0=x1_sb[:, qt, :], scalar1=mean2, scalar2=rstd2,
                    op0=mybir.AluOpType.subtract, op1=mybir.AluOpType.mult)
                for ct in range(CT):
                    tp2 = ps_t2.tile([P, P], bf16, tag="tp2")
                    nc.tensor.transpose(tp2, t2[:, ct * P:(ct + 1) * P], ident)
                    nc.vector.tensor_copy(out=ln2t[:, ct, q0:q0 + P], in_=tp2)
        sBC.close()  # free ot_sb

        # ---------------- Phase D: MLP ----------------
        with ExitStack() as sD:
            w1p = sD.enter_context(tc.tile_pool(name="w1_p", bufs=1))
            w2p = sD.enter_context(tc.tile_pool(name="w2_p", bufs=1))
            gtp = sD.enter_context(tc.tile_pool(name="gt_p", bufs=1))
            outp = sD.enter_context(tc.tile_pool(name="out_p", bufs=2))
            ps_f1 = sD.enter_context(tc.tile_pool(name="ps_f1", bufs=3, space="PSUM"))
            ps_f2 = sD.enter_context(tc.tile_pool(name="ps_f2", bufs=2, space="PSUM"))
            w1_sb = w1p.tile([P, CT, F], bf16)
            w1r = w1d.rearrange("(t p) f -> p t f", p=P)
            for kt in range(CT):  # chunked so fc1 can start on the first k-tile
                nc.sync.dma_start(out=w1_sb[:, kt, :], in_=w1r[:, kt, :])
            w2_sb = w2p.tile([P, FT, C], bf16)
            w2r = w2d.rearrange("(t p) o -> p t o", p=P)
            for kt in range(FT):
                nc.sync.dma_start(out=w2_sb[:, kt, :], in_=w2r[:, kt, :])
            gt_sb = gtp.tile([P, FT, RPC], bf16)
            for ft in range(FT):
                ps = ps_f1.tile([P, RPC], f32, tag="psf1")
                for kt in range(CT):
                    nc.tensor.matmul(
                        ps, lhsT=w1_sb[:, kt, ft * P:(ft + 1) * P],
                        rhs=ln2t[:, kt, :], start=(kt == 0), stop=(kt == CT - 1))
                nc.scalar.activation(
                    out=gt_sb[:, ft, :], in_=ps,
                    func=mybir.ActivationFunctionType.Gelu,
                    bias=b1_sb[:, ft:ft + 1], scale=1.0)
            for qt in range(QT):
                ps = ps_f2.tile([P, C], f32, tag="psf2")
                q0 = qt * P
                for c0, c1 in ((0, 512), (512, 768)):
                    for kt in range(FT):
                        nc.tensor.matmul(
                            ps[:, c0:c1], lhsT=gt_sb[:, kt, q0:q0 + P],
                            rhs=w2_sb[:, kt, c0:c1],
                            start=(kt == 0), stop=(kt == FT - 1))
                ot = outp.tile([P, C], f32, tag="ot")
                nc.vector.tensor_tensor(
                    out=ot, in0=ps, in1=x1_sb[:, qt, :], op=mybir.AluOpType.add)
                nc.vector.tensor_tensor(
                    out=ot, in0=ot, in1=b2_bc, op=mybir.AluOpType.add)
                nc.sync.dma_start(out=outd[q0:q0 + P, :], in_=ot)

    nc.compile()
    return nc


def prep_inputs(x, attn_mask, ln1_g, ln1_b, w_qkv, b_qkv, w_proj, b_proj, ls1,
                ln2_g, ln2_b, w1, b1, w2, b2, ls2):
    """Host-side folding + per-core slicing. Returns list of 8 in_maps."""
    f = np.float32
    x = np.asarray(x, f)
    wqkv_f = (np.asarray(ln1_g, f)[:, None] * np.asarray(w_qkv, f)).astype(BF16)
    bqkv_f = (np.asarray(b_qkv, f)
              + np.asarray(ln1_b, f) @ np.asarray(w_qkv, f)).astype(f)
    wproj_f = (np.asarray(w_proj, f) * np.asarray(ls1, f)[None, :]) \
        .reshape(H, D, C).transpose(1, 0, 2).astype(BF16).copy()
    bproj_f = (np.asarray(b_proj, f) * np.asarray(ls1, f)).astype(f)
    w1_f = (np.asarray(ln2_g, f)[:, None] * np.asarray(w1, f)).astype(BF16)
    b1_f = (np.asarray(b1, f) + np.asarray(ln2_b, f) @ np.asarray(w1, f)).astype(f)
    w2_f = (np.asarray(w2, f) * np.asarray(ls2, f)[None, :]).astype(BF16)
    b2_f = (np.asarray(b2, f) * np.asarray(ls2, f)).astype(f)
    allowedT = (~np.asarray(attn_mask)).T.astype(BF16)  # [key, q] 1/0

    in_maps = []
    for c in range(NCORES):
        b, r = c // GPB, c % GPB
        xb_c = np.roll(x[b], -RPC * r, axis=0).copy()
        maskt_c = np.roll(allowedT[:, RPC * r:RPC * (r + 1)], -RPC * r,
                          axis=0).copy()
        in_maps.append({
            "xb": xb_c, "maskt": maskt_c, "wqkv": wqkv_f, "bqkv": bqkv_f,
            "wproj": wproj_f, "bproj": bproj_f, "w1d": w1_f, "b1d": b1_f,
            "w2d": w2_f, "b2d": b2_f,
        })
    return in_maps


_NC_CACHE = None


def kernel(**inputs):
    global _NC_CACHE
    if _NC_CACHE is None:
        _NC_CACHE = build_nc()
    nc = _NC_CACHE
    in_maps = prep_inputs(**inputs)
    res = run_bass_kernel_spmd(nc, in_maps, core_ids=list(range(NCORES)))
    kernel.last_result = res
    out = np.empty((B, N, C), np.float32)
    for c in range(NCORES):
        b, r = c // GPB, c % GPB
        out[b, RPC * r:RPC * (r + 1)] = res.results[c]["out"]
    return out


# revision 22
# speedup vs baseline: 1.1365x; 1.1365x over previous
"""Fused transformer block (LN1 -> MHA -> LN2 -> MLP, layerscale residuals)
for 8 Trainium2 NeuronCores.

Sharding: core c in 0..7 owns batch b = c//4 and query-row quarter r = c%4
(512 rows of 2048).  Each core recomputes K/V for its whole batch (no
cross-core communication), computes attention + proj + MLP for its own 512
rows, and writes a [512, 768] fp32 output slice.  Host assembles the full
[2, 2048, 768] output.

Dataflow is kept transposed where it kills transposes:
  LN1^T [C, rows] -> Q^T/K^T [cols, rows] (via w_qkv as lhsT) and V row-major
  [rows, cols] (via LN1^T as lhsT).  Scores are computed as S^T [keys, q]
  (lhsT = K^T head slice, rhs = Q^T head slice), exp'd on ACT with the 1/8
  scale, masked multiplicatively (exact: masked -> 0), and fed straight into
  AV as the moving operand with a ones-augmented V as lhsT so the softmax
  denominator falls out of the matmul's 65th output partition.  o^T [d, q]
  then feeds proj as lhsT; fc1 produces g^T [4C, q] which feeds fc2 as lhsT.

Host-side folding: LN gains/biases are absorbed into w_qkv/w1 + biases,
layerscales into w_proj/w2 + biases, so the kernel only ever normalizes
(x - mu) * rsigma.
"""

from contextlib import ExitStack

import numpy as np
import ml_dtypes

import concourse.bacc as bacc
import concourse.bass as bass
import concourse.tile as tile
from concourse import mybir
from concourse.bass_utils import run_bass_kernel_spmd
from concourse.masks import make_identity

BF16 = ml_dtypes.bfloat16

B, N, C, H, D = 2, 2048, 768, 12, 64
F = 4 * C            # 3072
P = 128
NCORES = 8
GPB = NCORES // B    # cores (row-quarters) per batch = 4
RPC = N // GPB       # rows per core = 512
KT = N // P          # key tiles = 16
CT = C // P          # channel tiles = 6
FT = F // P          # mlp hidden tiles = 24
QT = RPC // P        # own-row tiles = 4
EPS = 1e-5

f32 = mybir.dt.float32
bf16 = mybir.dt.bfloat16


def _ln_stats(nc, pool, xt, eps_tile, rows=P):
    """Return (mean, rstd) [rows,1] f32 for xt [rows, C] f32."""
    xr = xt.rearrange("p (s g) -> p s g", g=256)
    stats = pool.tile([P, 3, nc.vector.BN_STATS_DIM], f32, tag="ln_stats")
    for s in range(3):
        nc.vector.bn_stats(out=stats[:rows, s, :], in_=xr[:rows, s, :])
    mv = pool.tile([P, nc.vector.BN_AGGR_DIM], f32, tag="ln_mv")
    nc.vector.bn_aggr(out=mv[:rows], in_=stats[:rows])
    mean = mv[:rows, 0:1]
    rstd = pool.tile([P, 1], f32, tag="ln_rstd")
    # rstd <- 1/sqrt(var + eps)
    nc.scalar.activation(
        out=rstd[:rows], in_=mv[:rows, 1:2],
        func=mybir.ActivationFunctionType.Sqrt,
        bias=eps_tile[:rows], scale=1.0,
    )
    nc.vector.reciprocal(out=rstd[:rows], in_=rstd[:rows])
    return mean, rstd


def _ln_normalize_transpose(nc, pool, ps_tp, ident, xt, mean, rstd, lnt, col0,
                            out_pairs=CT // 2):
    """tbf = (xt - mean) * rstd (gpsimd, ->bf16), PE-transpose 128x128 blocks,
    drain pairs of blocks into lnt[:, ct, col0:col0+128]."""
    tbf = pool.tile([P, C], bf16, tag="tbf")
    nc.vector.tensor_scalar(
        out=tbf, in0=xt, scalar1=mean, scalar2=rstd,
        op0=mybir.AluOpType.subtract, op1=mybir.AluOpType.mult)
    for cp in range(out_pairs):
        tp = ps_tp.tile([P, 2, P], bf16, tag="tp")
        for j in range(2):
            ct = cp * 2 + j
            nc.tensor.transpose(tp[:, j, :], tbf[:, ct * P:(ct + 1) * P], ident)
        nc.vector.tensor_copy(out=lnt[:, cp * 2:cp * 2 + 2, col0:col0 + P],
                              in_=tp)


def build_nc():
    nc = bacc.Bacc(None, target_bir_lowering=False)

    xb = nc.dram_tensor("xb", [N, C], f32, kind="ExternalInput")
    maskt = nc.dram_tensor("maskt", [N, RPC], bf16, kind="ExternalInput")
    wqkv = nc.dram_tensor("wqkv", [C, 3 * C], bf16, kind="ExternalInput")
    bqkv = nc.dram_tensor("bqkv", [3 * C], f32, kind="ExternalInput")
    wproj = nc.dram_tensor("wproj", [D, H, C], bf16, kind="ExternalInput")
    bproj = nc.dram_tensor("bproj", [C], f32, kind="ExternalInput")
    w1d = nc.dram_tensor("w1d", [C, F], bf16, kind="ExternalInput")
    b1d = nc.dram_tensor("b1d", [F], f32, kind="ExternalInput")
    w2d = nc.dram_tensor("w2d", [F, C], bf16, kind="ExternalInput")
    b2d = nc.dram_tensor("b2d", [C], f32, kind="ExternalInput")
    outd = nc.dram_tensor("out", [RPC, C], f32, kind="ExternalOutput")

    def bcast_ap(vec, n):
        # [n] dram vector -> [P, n] partition-broadcast AP
        if not isinstance(vec, bass.AP):
            vec = vec[:]
        return bass.AP(tensor=vec.tensor, offset=vec.offset,
                       ap=[[0, P], *vec.ap])

    with tile.TileContext(nc) as tc, ExitStack() as root:
        consts = root.enter_context(tc.tile_pool(name="consts", bufs=1))
        ident = consts.tile([P, P], bf16)
        make_identity(nc, ident)
        eps_tile = consts.tile([P, 1], f32)
        nc.vector.memset(eps_tile, EPS)
        bqkv_sb = consts.tile([P, 18], f32)
        nc.sync.dma_start(out=bqkv_sb, in_=bqkv.rearrange("(t p) -> p t", p=P))
        b1_sb = consts.tile([P, FT], f32)
        nc.sync.dma_start(out=b1_sb, in_=b1d.rearrange("(t p) -> p t", p=P))
        bproj_bc = consts.tile([P, C], f32)
        nc.gpsimd.dma_start(out=bproj_bc, in_=bcast_ap(bproj, C))
        b2_bc = consts.tile([P, C], f32)
        nc.gpsimd.dma_start(out=b2_bc, in_=bcast_ap(b2d, C))
        bv_bc = consts.tile([P, C], f32)
        nc.gpsimd.dma_start(out=bv_bc, in_=bcast_ap(bqkv[2 * C:3 * C], C))

        # Long-lived pools, created in reverse order of release (LIFO alloc):
        x1p = root.enter_context(tc.tile_pool(name="x1_p", bufs=1))
        ln2p = root.enter_context(tc.tile_pool(name="ln2t_p", bufs=1))
        x1_sb = x1p.tile([P, QT, C], f32)
        ln2t = ln2p.tile([P, CT, RPC], bf16)
        sBC = root.enter_context(ExitStack())
        otp = sBC.enter_context(tc.tile_pool(name="ot_p", bufs=1))
        ot_sb = otp.tile([D, H, RPC], bf16)

        # ---------------- Phase A: LN1 + Q^T/V (interleaved) --------------
        # Pools living through attention (freed before MLP):
        sAB = root.enter_context(ExitStack())
        qk_pool = sAB.enter_context(tc.tile_pool(name="qk_sb", bufs=1))
        vaug_pool = sAB.enter_context(tc.tile_pool(name="vaug_sb", bufs=1))
        wp = sAB.enter_context(tc.tile_pool(name="wqkv_p", bufs=1))
        lp = sAB.enter_context(tc.tile_pool(name="ln1t_p", bufs=1))
        ktp = sAB.enter_context(tc.tile_pool(name="kt_p", bufs=2))
        qt_sb = qk_pool.tile([P, CT, RPC], bf16)
        maskt_sb = qk_pool.tile([P, KT, RPC], bf16)
        nc.gpsimd.dma_start(out=maskt_sb,
                            in_=maskt.rearrange("(t p) q -> p t q", p=P))
        vaug = vaug_pool.tile([P, KT, H, D + 1], bf16)
        nc.vector.memset(vaug[:, :, :, D:D + 1], 1.0)
        wqkv_sb = wp.tile([P, CT, 3 * C], bf16)
        wqkvr = wqkv.rearrange("(t p) o -> p t o", p=P)
        for kt in range(CT):  # v columns first: V matmuls start earliest
            nc.gpsimd.dma_start(out=wqkv_sb[:, kt, 2 * C:3 * C],
                                in_=wqkvr[:, kt, 2 * C:3 * C])
        for kt in range(CT):
            nc.gpsimd.dma_start(out=wqkv_sb[:, kt, 0:2 * C],
                                in_=wqkvr[:, kt, 0:2 * C])
        ln1t = lp.tile([P, CT, N], bf16)

        def kt_chunk(dst, mt, nt, ps_pool, tag):
            """K^T chunk: cols (CT+mt)*128.., rows nt*512.. -> dst[:, chunk]."""
            ps = ps_pool.tile([P, 512], f32, tag=tag, name="ps_ktc")
            for kt in range(CT):
                nc.tensor.matmul(
                    ps, lhsT=wqkv_sb[:, kt, (CT + mt) * P:(CT + mt + 1) * P],
                    rhs=ln1t[:, kt, nt * 512:(nt + 1) * 512],
                    start=(kt == 0), stop=(kt == CT - 1))
            nc.vector.tensor_scalar(
                out=dst[:, nt * 512:(nt + 1) * 512], in0=ps,
                scalar1=bqkv_sb[:, CT + mt:CT + mt + 1], scalar2=None,
                op0=mybir.AluOpType.add)

        kt0_sb = ktp.tile([P, N], bf16, tag="kt", name="kt0_sb")
        with ExitStack() as sA:
            lnw = sA.enter_context(tc.tile_pool(name="ln1_work", bufs=3))
            ps_tp = sA.enter_context(tc.tile_pool(name="ps_tp", bufs=2, space="PSUM"))
            ps_v = sA.enter_context(tc.tile_pool(name="ps_v", bufs=2, space="PSUM"))
            ps_qt = sA.enter_context(tc.tile_pool(name="ps_qt", bufs=2, space="PSUM"))
            for rt in range(KT):
                xt = lnw.tile([P, C], f32, tag="xt")
                nc.sync.dma_start(out=xt, in_=xb[rt * P:(rt + 1) * P, :])
                mean, rstd = _ln_stats(nc, lnw, xt, eps_tile)
                _ln_normalize_transpose(nc, lnw, ps_tp, ident, xt, mean, rstd,
                                        ln1t, rt * P)
                # V for this key tile
                ps = ps_v.tile([P, C], f32, tag="psv")
                for c0, c1 in ((0, 512), (512, 768)):
                    for kt in range(CT):
                        nc.tensor.matmul(
                            ps[:, c0:c1],
                            lhsT=ln1t[:, kt, rt * P:(rt + 1) * P],
                            rhs=wqkv_sb[:, kt, 2 * C + c0:2 * C + c1],
                            start=(kt == 0), stop=(kt == CT - 1))
                nc.vector.tensor_tensor(
                    out=vaug[:, rt, :, 0:D],
                    in0=ps.rearrange("p (h d) -> p h d", d=D),
                    in1=bv_bc.rearrange("p (h d) -> p h d", d=D),
                    op=mybir.AluOpType.add)
                if rt == 3:
                    # own rows done -> Q^T for all heads
                    for mt in range(CT):
                        ps = ps_qt.tile([P, RPC], f32, tag="psqt")
                        for kt in range(CT):
                            nc.tensor.matmul(
                                ps, lhsT=wqkv_sb[:, kt, mt * P:(mt + 1) * P],
                                rhs=ln1t[:, kt, 0:RPC],
                                start=(kt == 0), stop=(kt == CT - 1))
                        nc.vector.tensor_scalar(
                            out=qt_sb[:, mt, :], in0=ps,
                            scalar1=bqkv_sb[:, mt:mt + 1], scalar2=None,
                            op0=mybir.AluOpType.add)
                if rt % 4 == 3:
                    # K^T head-pair 0, row chunk rt//4 (rows just completed)
                    kt_chunk(kt0_sb, 0, rt // 4, ps_qt, "psqt")

        # ------------- Phase B: fused K^T generation + attention ----------
        # Per mt (head pair): scores(2mt), scores(2mt+1), K^T(mt+1), AV pair.
        # The K^T matmuls keep PE busy while ACT runs the exps.
        with ExitStack() as sB:
            ptp = sB.enter_context(tc.tile_pool(name="pt_p", bufs=3))
            recp = sB.enter_context(tc.tile_pool(name="rec_p", bufs=2))
            dramp = sB.enter_context(tc.tile_pool(name="den_dram", bufs=2,
                                                  space="DRAM"))
            ps_kt = sB.enter_context(tc.tile_pool(name="ps_kt", bufs=2, space="PSUM"))
            ps_s = sB.enter_context(tc.tile_pool(name="ps_s", bufs=2, space="PSUM"))
            ps_av = sB.enter_context(tc.tile_pool(name="ps_av", bufs=2, space="PSUM"))
            kt_tiles = {0: kt0_sb}

            def scores_group(h, g, pt):
                po = D * (h % 2)
                mt = h // 2
                sps = ps_s.tile([P, 2, RPC], f32, tag="sps", name="sps")
                for j in range(2):
                    k0 = (g * 2 + j) * P
                    nc.tensor.matmul(
                        sps[:, j, :],
                        lhsT=kt_tiles[mt][po:po + D, k0:k0 + P],
                        rhs=qt_sb[po:po + D, mt, :],
                        start=True, stop=True)
                nc.scalar.activation(
                    out=pt[:, g * 2:g * 2 + 2, :], in_=sps,
                    func=mybir.ActivationFunctionType.Exp,
                    scale=float(D) ** -0.5)
                nc.vector.tensor_tensor(
                    out=pt[:, g * 2:g * 2 + 2, :],
                    in0=pt[:, g * 2:g * 2 + 2, :],
                    in1=maskt_sb[:, g * 2:g * 2 + 2, :],
                    op=mybir.AluOpType.mult)

            def av_part(h, avps, pt, g):
                for j in range(2):
                    kt = g * 2 + j
                    nc.tensor.matmul(
                        avps, lhsT=vaug[:, kt, h, :], rhs=pt[:, kt, :],
                        start=(kt == 0), stop=(kt == KT - 1))

            def finish_head(h, avps):
                den = recp.tile([D + 1, RPC], f32, tag="den", name="den")
                nc.vector.tensor_copy(out=den[D:D + 1, :], in_=avps[D:D + 1, :])
                nc.vector.reciprocal(out=den[D:D + 1, :], in_=den[D:D + 1, :])
                # broadcast to 64 partitions via a DRAM bounce (DRAM sources
                # allow 0-stride partition APs; SBUF ones don't)
                dscr = dramp.tile([1, RPC], f32, name="dscr")
                nc.gpsimd.dma_start(out=dscr[:, :], in_=den[D:D + 1, :])
                den_full = recp.tile([D, RPC], f32, tag="den_bc", name="den_full")
                nc.gpsimd.dma_start(
                    out=den_full,
                    in_=bass.AP(tensor=dscr.tensor, offset=dscr.offset,
                                ap=[[0, D], [1, RPC]]))
                nc.vector.tensor_tensor(
                    out=ot_sb[:, h, :], in0=avps[0:D, :], in1=den_full,
                    op=mybir.AluOpType.mult)

            # One-head-delayed pipeline: scores(h) weave with AV(h-1) and the
            # next head pair's K^T chunks, so the PE FIFO never waits on ACT.
            pt_prev = None
            for h in range(H):
                pt = ptp.tile([P, KT, RPC], bf16, tag="pt", name="pt")
                avps = None
                for g in range(KT // 2):
                    scores_group(h, g, pt)
                    if h % 2 == 1 and g in (1, 3, 5, 7) and h // 2 + 1 < CT:
                        p_next = h // 2 + 1
                        if p_next not in kt_tiles:
                            kt_tiles[p_next] = ktp.tile([P, N], bf16, tag="kt",
                                                        name="kt_sb_n")
                        kt_chunk(kt_tiles[p_next], p_next, g // 2, ps_kt, "psktc")
                    if pt_prev is not None:
                        if avps is None:
                            avps = ps_av.tile([D + 1, RPC], f32, tag="avps",
                                              name="avps")
                        av_part(h - 1, avps, pt_prev, g)
                if pt_prev is not None:
                    finish_head(h - 1, avps)
                pt_prev = pt
            # epilogue: AV + normalize for the last head
            avps = ps_av.tile([D + 1, RPC], f32, tag="avps", name="avps")
            for g in range(KT // 2):
                av_part(H - 1, avps, pt_prev, g)
            finish_head(H - 1, avps)
        sAB.close()  # free qt/kt/vaug/wqkv before the MLP weights arrive

        # ---------------- Phase C: proj + residual + LN2 ----------------
        with ExitStack() as sC:
            wpp = sC.enter_context(tc.tile_pool(name="wproj_p", bufs=1))
            xqp = sC.enter_context(tc.tile_pool(name="xq_p", bufs=2))
            ln2w = sC.enter_context(tc.tile_pool(name="ln2_work", bufs=3))
            ps_pj = sC.enter_context(tc.tile_pool(name="ps_pj", bufs=2, space="PSUM"))
            ps_t2 = sC.enter_context(tc.tile_pool(name="ps_t2", bufs=2, space="PSUM"))
            wproj_sb = wpp.tile([D, H, C], bf16)
            nc.gpsimd.dma_start(out=wproj_sb, in_=wproj[:, :, :])
            for qt in range(QT):
                ps = ps_pj.tile([P, C], f32, tag="pspj")
                q0 = qt * P
                for c0, c1 in ((0, 512), (512, 768)):
                    for h in range(H):
                        nc.tensor.matmul(
                            ps[:, c0:c1], lhsT=ot_sb[:, h, q0:q0 + P],
                            rhs=wproj_sb[:, h, c0:c1],
                            start=(h == 0), stop=(h == H - 1))
                xq = xqp.tile([P, C], f32, tag="xq")
                nc.sync.dma_start(out=xq, in_=xb[q0:q0 + P, :])
                nc.vector.tensor_tensor(
                    out=x1_sb[:, qt, :], in0=ps, in1=xq, op=mybir.AluOpType.add)
                nc.vector.tensor_tensor(
                    out=x1_sb[:, qt, :], in0=x1_sb[:, qt, :], in1=bproj_bc,
                    op=mybir.AluOpType.add)
                mean2, rstd2 = _ln_stats(nc, ln2w, x1_sb[:, qt, :], eps_tile)
                t2 = ln2w.tile([P, C], bf16, tag="t2")
                nc.vector.tensor_scalar(
                    out=t2, in0=x1_sb[:, qt, :], scalar1=mean2, scalar2=rstd2,
                    op0=mybir.AluOpType.subtract, op1=mybir.AluOpType.mult)
                for ct in range(CT):
                    tp2 = ps_t2.tile([P, P], bf16, tag="tp2")
                    nc.tensor.transpose(tp2, t2[:, ct * P:(ct + 1) * P], ident)
                    nc.vector.tensor_copy(out=ln2t[:, ct, q0:q0 + P], in_=tp2)
        sBC.close()  # free ot_sb

        # ---------------- Phase D: MLP ----------------
        with ExitStack() as sD:
            w1p = sD.enter_context(tc.tile_pool(name="w1_p", bufs=1))
            w2p = sD.enter_context(tc.tile_pool(name="w2_p", bufs=1))
            gtp = sD.enter_context(tc.tile_pool(name="gt_p", bufs=1))
            outp = sD.enter_context(tc.tile_pool(name="out_p", bufs=2))
            ps_f1 = sD.enter_context(tc.tile_pool(name="ps_f1", bufs=3, space="PSUM"))
            ps_f2 = sD.enter_context(tc.tile_pool(name="ps_f2", bufs=2, space="PSUM"))
            w1_sb = w1p.tile([P, CT, F], bf16)
            w1r = w1d.rearrange("(t p) f -> p t f", p=P)
            for kt in range(CT):  # chunked so fc1 can start on the first k-tile
                nc.gpsimd.dma_start(out=w1_sb[:, kt, :], in_=w1r[:, kt, :])
            w2_sb = w2p.tile([P, FT, C], bf16)
            w2r = w2d.rearrange("(t p) o -> p t o", p=P)
            for kt in range(FT):
                nc.gpsimd.dma_start(out=w2_sb[:, kt, :], in_=w2r[:, kt, :])
            gt_sb = gtp.tile([P, FT, RPC], bf16)
            for ft in range(FT):
                ps = ps_f1.tile([P, RPC], f32, tag="psf1")
                for kt in range(CT):
                    nc.tensor.matmul(
                        ps, lhsT=w1_sb[:, kt, ft * P:(ft + 1) * P],
                        rhs=ln2t[:, kt, :], start=(kt == 0), stop=(kt == CT - 1))
                nc.scalar.activation(
                    out=gt_sb[:, ft, :], in_=ps,
                    func=mybir.ActivationFunctionType.Gelu,
                    bias=b1_sb[:, ft:ft + 1], scale=1.0)
            for qt in range(QT):
                ps = ps_f2.tile([P, C], f32, tag="psf2")
                q0 = qt * P
                for c0, c1 in ((0, 512), (512, 768)):
                    for kt in range(FT):
                        nc.tensor.matmul(
                            ps[:, c0:c1], lhsT=gt_sb[:, kt, q0:q0 + P],
                            rhs=w2_sb[:, kt, c0:c1],
                            start=(kt == 0), stop=(kt == FT - 1))
                ot = outp.tile([P, C], f32, tag="ot")
                nc.vector.tensor_tensor(
                    out=ot, in0=ps, in1=x1_sb[:, qt, :], op=mybir.AluOpType.add)
                nc.vector.tensor_tensor(
                    out=ot, in0=ot, in1=b2_bc, op=mybir.AluOpType.add)
                nc.sync.dma_start(out=outd[q0:q0 + P, :], in_=ot)

    nc.compile()
    return nc


def prep_inputs(x, attn_mask, ln1_g, ln1_b, w_qkv, b_qkv, w_proj, b_proj, ls1,
                ln2_g, ln2_b, w1, b1, w2, b2, ls2):
    """Host-side folding + per-core slicing. Returns list of 8 in_maps."""
    f = np.float32
    x = np.asarray(x, f)
    wqkv_f = (np.asarray(ln1_g, f)[:, None] * np.asarray(w_qkv, f)).astype(BF16)
    bqkv_f = (np.asarray(b_qkv, f)
              + np.asarray(ln1_b, f) @ np.asarray(w_qkv, f)).astype(f)
    wproj_f = (np.asarray(w_proj, f) * np.asarray(ls1, f)[None, :]) \
        .reshape(H, D, C).transpose(1, 0, 2).astype(BF16).copy()
    bproj_f = (np.asarray(b_proj, f) * np.asarray(ls1, f)).astype(f)
    w1_f = (np.asarray(ln2_g, f)[:, None] * np.asarray(w1, f)).astype(BF16)
    b1_f = (np.asarray(b1, f) + np.asarray(ln2_b, f) @ np.asarray(w1, f)).astype(f)
    w2_f = (np.asarray(w2, f) * np.asarray(ls2, f)[None, :]).astype(BF16)
    b2_f = (np.asarray(b2, f) * np.asarray(ls2, f)).astype(f)
    allowedT = (~np.asarray(attn_mask)).T.astype(BF16)  # [key, q] 1/0

    in_maps = []
    for c in range(NCORES):
        b, r = c // GPB, c % GPB
        xb_c = np.roll(x[b], -RPC * r, axis=0).copy()
        maskt_c = np.roll(allowedT[:, RPC * r:RPC * (r + 1)], -RPC * r,
                          axis=0).copy()
        in_maps.append({
            "xb": xb_c, "maskt": maskt_c, "wqkv": wqkv_f, "bqkv": bqkv_f,
            "wproj": wproj_f, "bproj": bproj_f, "w1d": w1_f, "b1d": b1_f,
            "w2d": w2_f, "b2d": b2_f,
        })
    return in_maps


_NC_CACHE = None


def kernel(**inputs):
    global _NC_CACHE
    if _NC_CACHE is None:
        _NC_CACHE = build_nc()
    nc = _NC_CACHE
    in_maps = prep_inputs(**inputs)
    res = run_bass_kernel_spmd(nc, in_maps, core_ids=list(range(NCORES)))
    kernel.last_result = res
    out = np.empty((B, N, C), np.float32)
    for c in range(NCORES):
        b, r = c // GPB, c % GPB
        out[b, RPC * r:RPC * (r + 1)] = res.results[c]["out"]
    return out


# revision 29
# speedup vs baseline: 1.2156x; 1.0696x over previous
"""Fused transformer block (LN1 -> MHA -> LN2 -> MLP, layerscale residuals)
for 8 Trainium2 NeuronCores.

Sharding: core c in 0..7 owns batch b = c//4 and query-row quarter r = c%4
(512 rows of 2048).  Each core recomputes K/V for its whole batch (no
cross-core communication), computes attention + proj + MLP for its own 512
rows, and writes a [512, 768] fp32 output slice.  Host assembles the full
[2, 2048, 768] output.

Dataflow is kept transposed where it kills transposes:
  LN1^T [C, rows] -> Q^T/K^T [cols, rows] (via w_qkv as lhsT) and V row-major
  [rows, cols] (via LN1^T as lhsT).  Scores are computed as S^T [keys, q]
  (lhsT = K^T head slice, rhs = Q^T head slice), exp'd on ACT with the 1/8
  scale, masked multiplicatively (exact: masked -> 0), and fed straight into
  AV as the moving operand with a ones-augmented V as lhsT so the softmax
  denominator falls out of the matmul's 65th output partition.  o^T [d, q]
  then feeds proj as lhsT; fc1 produces g^T [4C, q] which feeds fc2 as lhsT.

Host-side folding: LN gains/biases are absorbed into w_qkv/w1 + biases,
layerscales into w_proj/w2 + biases, so the kernel only ever normalizes
(x - mu) * rsigma.
"""

from contextlib import ExitStack

import numpy as np
import ml_dtypes

import concourse.bacc as bacc
import concourse.bass as bass
import concourse.tile as tile
from concourse import mybir
from concourse.bass_utils import run_bass_kernel_spmd
from concourse.masks import make_identity

BF16 = ml_dtypes.bfloat16

B, N, C, H, D = 2, 2048, 768, 12, 64
F = 4 * C            # 3072
P = 128
NCORES = 8
GPB = NCORES // B    # cores (row-quarters) per batch = 4
RPC = N // GPB       # rows per core = 512
KT = N // P          # key tiles = 16
CT = C // P          # channel tiles = 6
FT = F // P          # mlp hidden tiles = 24
QT = RPC // P        # own-row tiles = 4
EPS = 1e-5

f32 = mybir.dt.float32
bf16 = mybir.dt.bfloat16


def _ln_stats(nc, pool, xt, eps_tile, rows=P):
    """Return (mean, rstd) [rows,1] f32 for xt [rows, C] f32."""
    xr = xt.rearrange("p (s g) -> p s g", g=256)
    stats = pool.tile([P, 3, nc.vector.BN_STATS_DIM], f32, tag="ln_stats")
    for s in range(3):
        nc.vector.bn_stats(out=stats[:rows, s, :], in_=xr[:rows, s, :])
    mv = pool.tile([P, nc.vector.BN_AGGR_DIM], f32, tag="ln_mv")
    nc.vector.bn_aggr(out=mv[:rows], in_=stats[:rows])
    mean = mv[:rows, 0:1]
    rstd = pool.tile([P, 1], f32, tag="ln_rstd")
    # rstd <- 1/sqrt(var + eps)
    nc.scalar.activation(
        out=rstd[:rows], in_=mv[:rows, 1:2],
        func=mybir.ActivationFunctionType.Sqrt,
        bias=eps_tile[:rows], scale=1.0,
    )
    nc.vector.reciprocal(out=rstd[:rows], in_=rstd[:rows])
    return mean, rstd


def _ln_normalize_transpose(nc, pool, ps_tp, ident, xt, mean, rstd, lnt, col0,
                            out_pairs=CT // 2):
    """tbf = (xt - mean) * rstd (gpsimd, ->bf16), PE-transpose 128x128 blocks,
    drain pairs of blocks into lnt[:, ct, col0:col0+128]."""
    tbf = pool.tile([P, C], bf16, tag="tbf")
    nc.vector.tensor_scalar(
        out=tbf, in0=xt, scalar1=mean, scalar2=rstd,
        op0=mybir.AluOpType.subtract, op1=mybir.AluOpType.mult)
    for cp in range(out_pairs):
        tp = ps_tp.tile([P, 2, P], bf16, tag="tp")
        for j in range(2):
            ct = cp * 2 + j
            nc.tensor.transpose(tp[:, j, :], tbf[:, ct * P:(ct + 1) * P], ident)
        nc.vector.tensor_copy(out=lnt[:, cp * 2:cp * 2 + 2, col0:col0 + P],
                              in_=tp)


def build_nc():
    nc = bacc.Bacc(None, target_bir_lowering=False)

    xb = nc.dram_tensor("xb", [N, C], f32, kind="ExternalInput")
    maskt = nc.dram_tensor("maskt", [N, RPC], bf16, kind="ExternalInput")
    wqkv = nc.dram_tensor("wqkv", [C, 3 * C], bf16, kind="ExternalInput")
    bqkv = nc.dram_tensor("bqkv", [3 * C], f32, kind="ExternalInput")
    wproj = nc.dram_tensor("wproj", [D, H, C], bf16, kind="ExternalInput")
    bproj = nc.dram_tensor("bproj", [C], f32, kind="ExternalInput")
    w1d = nc.dram_tensor("w1d", [C, F], bf16, kind="ExternalInput")
    b1d = nc.dram_tensor("b1d", [F], f32, kind="ExternalInput")
    w2d = nc.dram_tensor("w2d", [F, C], bf16, kind="ExternalInput")
    b2d = nc.dram_tensor("b2d", [C], f32, kind="ExternalInput")
    outd = nc.dram_tensor("out", [RPC, C], f32, kind="ExternalOutput")

    def bcast_ap(vec, n):
        # [n] dram vector -> [P, n] partition-broadcast AP
        if not isinstance(vec, bass.AP):
            vec = vec[:]
        return bass.AP(tensor=vec.tensor, offset=vec.offset,
                       ap=[[0, P], *vec.ap])

    with tile.TileContext(nc) as tc, ExitStack() as root:
        consts = root.enter_context(tc.tile_pool(name="consts", bufs=1))
        ident = consts.tile([P, P], bf16)
        make_identity(nc, ident)
        eps_tile = consts.tile([P, 1], f32)
        nc.vector.memset(eps_tile, EPS)
        bqkv_sb = consts.tile([P, 18], f32)
        nc.sync.dma_start(out=bqkv_sb, in_=bqkv.rearrange("(t p) -> p t", p=P))
        b1_sb = consts.tile([P, FT], f32)
        nc.sync.dma_start(out=b1_sb, in_=b1d.rearrange("(t p) -> p t", p=P))
        ones1 = consts.tile([1, P], f32)
        nc.vector.memset(ones1, 1.0)
        bprow = consts.tile([1, C], f32)
        nc.gpsimd.dma_start(out=bprow, in_=bcast_ap(bproj, C)[0:1, :])
        b2row = consts.tile([1, C], f32)
        nc.gpsimd.dma_start(out=b2row, in_=bcast_ap(b2d, C)[0:1, :])
        bv_bc = consts.tile([P, C], f32)
        nc.gpsimd.dma_start(out=bv_bc, in_=bcast_ap(bqkv[2 * C:3 * C], C))

        # Long-lived pools, created in reverse order of release (LIFO alloc):
        x1p = root.enter_context(tc.tile_pool(name="x1_p", bufs=1))
        ln2p = root.enter_context(tc.tile_pool(name="ln2t_p", bufs=1))
        x1_sb = x1p.tile([P, QT, C], f32)
        ln2t = ln2p.tile([P, CT, RPC], bf16)
        sBC = root.enter_context(ExitStack())
        otp = sBC.enter_context(tc.tile_pool(name="ot_p", bufs=1))
        ot_sb = otp.tile([D, H, RPC], bf16)

        # ---------------- Phase A: LN1 + Q^T/V (interleaved) --------------
        # Pools living through attention (freed before MLP):
        sAB = root.enter_context(ExitStack())
        qk_pool = sAB.enter_context(tc.tile_pool(name="qk_sb", bufs=1))
        vaug_pool = sAB.enter_context(tc.tile_pool(name="vaug_sb", bufs=1))
        wp = sAB.enter_context(tc.tile_pool(name="wqkv_p", bufs=1))
        lp = sAB.enter_context(tc.tile_pool(name="ln1t_p", bufs=1))
        ktp = sAB.enter_context(tc.tile_pool(name="kt_p", bufs=2))
        qt_sb = qk_pool.tile([P, CT, RPC], bf16)
        maskt_sb = qk_pool.tile([P, KT, RPC], bf16)
        vaug = vaug_pool.tile([P, KT, H, D + 1], bf16)
        nc.vector.memset(vaug[:, :, :, D:D + 1], 1.0)
        wqkv_sb = wp.tile([P, CT, 3 * C], bf16)
        wqkvr = wqkv.rearrange("(t p) o -> p t o", p=P)
        for kt in range(CT):  # v columns first: V matmuls start earliest
            nc.gpsimd.dma_start(out=wqkv_sb[:, kt, 2 * C:3 * C],
                                in_=wqkvr[:, kt, 2 * C:3 * C])
        for kt in range(CT):
            nc.gpsimd.dma_start(out=wqkv_sb[:, kt, 0:2 * C],
                                in_=wqkvr[:, kt, 0:2 * C])
        ln1t = lp.tile([P, CT, N], bf16)

        def kt_chunk(dst, mt, nt, ps_pool, tag):
            """K^T chunk: cols (CT+mt)*128.., rows nt*512.. -> dst[:, chunk]."""
            ps = ps_pool.tile([P, 512], f32, tag=tag, name="ps_ktc")
            for kt in range(CT):
                nc.tensor.matmul(
                    ps, lhsT=wqkv_sb[:, kt, (CT + mt) * P:(CT + mt + 1) * P],
                    rhs=ln1t[:, kt, nt * 512:(nt + 1) * 512],
                    start=(kt == 0), stop=(kt == CT - 1))
            nc.vector.tensor_scalar(
                out=dst[:, nt * 512:(nt + 1) * 512], in0=ps,
                scalar1=bqkv_sb[:, CT + mt:CT + mt + 1], scalar2=None,
                op0=mybir.AluOpType.add)

        kt0_sb = ktp.tile([P, N], bf16, tag="kt", name="kt0_sb")
        with ExitStack() as sA:
            lnw = sA.enter_context(tc.tile_pool(name="ln1_work", bufs=3))
            ps_tp = sA.enter_context(tc.tile_pool(name="ps_tp", bufs=2, space="PSUM"))
            ps_v = sA.enter_context(tc.tile_pool(name="ps_v", bufs=2, space="PSUM"))
            ps_qt = sA.enter_context(tc.tile_pool(name="ps_qt", bufs=2, space="PSUM"))
            for rt in range(KT):
                xt = lnw.tile([P, C], f32, tag="xt")
                nc.sync.dma_start(out=xt, in_=xb[rt * P:(rt + 1) * P, :])
                mean, rstd = _ln_stats(nc, lnw, xt, eps_tile)
                _ln_normalize_transpose(nc, lnw, ps_tp, ident, xt, mean, rstd,
                                        ln1t, rt * P)
                # V for this key tile
                ps = ps_v.tile([P, C], f32, tag="psv")
                for c0, c1 in ((0, 512), (512, 768)):
                    for kt in range(CT):
                        nc.tensor.matmul(
                            ps[:, c0:c1],
                            lhsT=ln1t[:, kt, rt * P:(rt + 1) * P],
                            rhs=wqkv_sb[:, kt, 2 * C + c0:2 * C + c1],
                            start=(kt == 0), stop=(kt == CT - 1))
                nc.vector.tensor_tensor(
                    out=vaug[:, rt, :, 0:D],
                    in0=ps.rearrange("p (h d) -> p h d", d=D),
                    in1=bv_bc.rearrange("p (h d) -> p h d", d=D),
                    op=mybir.AluOpType.add)
                if rt == 3:
                    # own rows done -> Q^T for all heads
                    for mt in range(CT):
                        ps = ps_qt.tile([P, RPC], f32, tag="psqt")
                        for kt in range(CT):
                            nc.tensor.matmul(
                                ps, lhsT=wqkv_sb[:, kt, mt * P:(mt + 1) * P],
                                rhs=ln1t[:, kt, 0:RPC],
                                start=(kt == 0), stop=(kt == CT - 1))
                        nc.vector.tensor_scalar(
                            out=qt_sb[:, mt, :], in0=ps,
                            scalar1=bqkv_sb[:, mt:mt + 1], scalar2=None,
                            op0=mybir.AluOpType.add)
                if rt % 4 == 3:
                    # K^T head-pair 0, row chunk rt//4 (rows just completed)
                    kt_chunk(kt0_sb, 0, rt // 4, ps_qt, "psqt")

        # ------------- Phase B: fused K^T generation + attention ----------
        # Per mt (head pair): scores(2mt), scores(2mt+1), K^T(mt+1), AV pair.
        # The K^T matmuls keep PE busy while ACT runs the exps.
        nc.gpsimd.dma_start(out=maskt_sb,  # late: don't contend with xt loads
                            in_=maskt.rearrange("(t p) q -> p t q", p=P))
        with ExitStack() as sB:
            ptp = sB.enter_context(tc.tile_pool(name="pt_p", bufs=3))
            recp = sB.enter_context(tc.tile_pool(name="rec_p", bufs=2))
            dramp = sB.enter_context(tc.tile_pool(name="den_dram", bufs=2,
                                                  space="DRAM"))
            ps_kt = sB.enter_context(tc.tile_pool(name="ps_kt", bufs=2, space="PSUM"))
            ps_s = sB.enter_context(tc.tile_pool(name="ps_s", bufs=2, space="PSUM"))
            ps_av = sB.enter_context(tc.tile_pool(name="ps_av", bufs=2, space="PSUM"))
            kt_tiles = {0: kt0_sb}

            def scores_group(h, g, pt):
                po = D * (h % 2)
                mt = h // 2
                sps = ps_s.tile([P, 2, RPC], f32, tag="sps", name="sps")
                for j in range(2):
                    k0 = (g * 2 + j) * P
                    nc.tensor.matmul(
                        sps[:, j, :],
                        lhsT=kt_tiles[mt][po:po + D, k0:k0 + P],
                        rhs=qt_sb[po:po + D, mt, :],
                        start=True, stop=True)
                nc.scalar.activation(
                    out=pt[:, g * 2:g * 2 + 2, :], in_=sps,
                    func=mybir.ActivationFunctionType.Exp,
                    scale=float(D) ** -0.5)
                nc.vector.tensor_tensor(
                    out=pt[:, g * 2:g * 2 + 2, :],
                    in0=pt[:, g * 2:g * 2 + 2, :],
                    in1=maskt_sb[:, g * 2:g * 2 + 2, :],
                    op=mybir.AluOpType.mult)

            def av_part(h, avps, pt, g):
                for j in range(2):
                    kt = g * 2 + j
                    nc.tensor.matmul(
                        avps, lhsT=vaug[:, kt, h, :], rhs=pt[:, kt, :],
                        start=(kt == 0), stop=(kt == KT - 1))

            def finish_head(h, avps):
                # denominator -> 1/denominator -> broadcast over 64 partitions.
                # The exact vector.reciprocal is per-lane serial, so bounce the
                # [1,512] row through DRAM reshaped to [128,4] to use all
                # lanes, then bounce back with a 0-stride partition broadcast
                # (legal for DRAM sources only).
                den = recp.tile([D + 1, RPC], f32, tag="den", name="den")
                nc.vector.tensor_copy(out=den[D:D + 1, :], in_=avps[D:D + 1, :])
                dscr = dramp.tile([1, RPC], f32, name="dscr")
                nc.gpsimd.dma_start(out=dscr[:, :], in_=den[D:D + 1, :])
                rsm = recp.tile([P, RPC // P], f32, tag="rsm", name="rsm")
                nc.gpsimd.dma_start(
                    out=rsm,
                    in_=bass.AP(tensor=dscr.tensor, offset=dscr.offset,
                                ap=[[RPC // P, P], [1, RPC // P]]))
                nc.vector.reciprocal(out=rsm, in_=rsm)
                dscr2 = dramp.tile([1, RPC], f32, name="dscr2")
                nc.gpsimd.dma_start(
                    out=bass.AP(tensor=dscr2.tensor, offset=dscr2.offset,
                                ap=[[RPC // P, P], [1, RPC // P]]),
                    in_=rsm)
                den_full = recp.tile([D, RPC], f32, tag="den_bc", name="den_full")
                nc.gpsimd.dma_start(
                    out=den_full,
                    in_=bass.AP(tensor=dscr2.tensor, offset=dscr2.offset,
                                ap=[[0, D], [1, RPC]]))
                nc.vector.tensor_tensor(
                    out=ot_sb[:, h, :], in0=avps[0:D, :], in1=den_full,
                    op=mybir.AluOpType.mult)

            # One-head-delayed pipeline: scores(h) weave with AV(h-1) and the
            # next head pair's K^T chunks, so the PE FIFO never waits on ACT.
            pt_prev = None
            for h in range(H):
                pt = ptp.tile([P, KT, RPC], bf16, tag="pt", name="pt")
                avps = None
                for g in range(KT // 2):
                    scores_group(h, g, pt)
                    if h % 2 == 1 and g in (1, 3, 5, 7) and h // 2 + 1 < CT:
                        p_next = h // 2 + 1
                        if p_next not in kt_tiles:
                            kt_tiles[p_next] = ktp.tile([P, N], bf16, tag="kt",
                                                        name="kt_sb_n")
                        kt_chunk(kt_tiles[p_next], p_next, g // 2, ps_kt, "psktc")
                    if pt_prev is not None:
                        if avps is None:
                            avps = ps_av.tile([D + 1, RPC], f32, tag="avps",
                                              name="avps")
                        av_part(h - 1, avps, pt_prev, g)
                if pt_prev is not None:
                    finish_head(h - 1, avps)
                pt_prev = pt
            # epilogue: AV + normalize for the last head
            avps = ps_av.tile([D + 1, RPC], f32, tag="avps", name="avps")
            for g in range(KT // 2):
                av_part(H - 1, avps, pt_prev, g)
            finish_head(H - 1, avps)
        sAB.close()  # free qt/kt/vaug/wqkv before the MLP weights arrive

        # ---------------- Phase C: proj + residual + LN2 ----------------
        with ExitStack() as sC:
            wpp = sC.enter_context(tc.tile_pool(name="wproj_p", bufs=1))
            xqp = sC.enter_context(tc.tile_pool(name="xq_p", bufs=2))
            ln2w = sC.enter_context(tc.tile_pool(name="ln2_work", bufs=3))
            ps_pj = sC.enter_context(tc.tile_pool(name="ps_pj", bufs=2, space="PSUM"))
            ps_t2 = sC.enter_context(tc.tile_pool(name="ps_t2", bufs=2, space="PSUM"))
            wproj_sb = wpp.tile([D, H, C], bf16)
            nc.gpsimd.dma_start(out=wproj_sb, in_=wproj[:, :, :])
            for qt in range(QT):
                ps = ps_pj.tile([P, C], f32, tag="pspj")
                q0 = qt * P
                for c0, c1 in ((0, 512), (512, 768)):
                    nc.tensor.matmul(  # bias row -> psum (start of group)
                        ps[:, c0:c1], lhsT=ones1[0:1, :],
                        rhs=bprow[0:1, c0:c1], start=True, stop=False)
                    for h in range(H):
                        nc.tensor.matmul(
                            ps[:, c0:c1], lhsT=ot_sb[:, h, q0:q0 + P],
                            rhs=wproj_sb[:, h, c0:c1],
                            start=False, stop=(h == H - 1))
                xq = xqp.tile([P, C], f32, tag="xq")
                nc.sync.dma_start(out=xq, in_=xb[q0:q0 + P, :])
                nc.vector.tensor_tensor(
                    out=x1_sb[:, qt, :], in0=ps, in1=xq, op=mybir.AluOpType.add)
                mean2, rstd2 = _ln_stats(nc, ln2w, x1_sb[:, qt, :], eps_tile)
                t2 = ln2w.tile([P, C], bf16, tag="t2")
                nc.vector.tensor_scalar(
                    out=t2, in0=x1_sb[:, qt, :], scalar1=mean2, scalar2=rstd2,
                    op0=mybir.AluOpType.subtract, op1=mybir.AluOpType.mult)
                for ct in range(CT):
                    tp2 = ps_t2.tile([P, P], bf16, tag="tp2")
                    nc.tensor.transpose(tp2, t2[:, ct * P:(ct + 1) * P], ident)
                    nc.vector.tensor_copy(out=ln2t[:, ct, q0:q0 + P], in_=tp2)
        sBC.close()  # free ot_sb

        # ---------------- Phase D: MLP ----------------
        with ExitStack() as sD:
            w1p = sD.enter_context(tc.tile_pool(name="w1_p", bufs=1))
            w2p = sD.enter_context(tc.tile_pool(name="w2_p", bufs=1))
            gtp = sD.enter_context(tc.tile_pool(name="gt_p", bufs=1))
            outp = sD.enter_context(tc.tile_pool(name="out_p", bufs=2))
            ps_f1 = sD.enter_context(tc.tile_pool(name="ps_f1", bufs=3, space="PSUM"))
            ps_f2 = sD.enter_context(tc.tile_pool(name="ps_f2", bufs=2, space="PSUM"))
            w1_sb = w1p.tile([P, CT, F], bf16)
            w1r = w1d.rearrange("(t p) f -> p t f", p=P)
            for kt in range(CT):  # chunked so fc1 can start on the first k-tile
                nc.gpsimd.dma_start(out=w1_sb[:, kt, :], in_=w1r[:, kt, :])
            w2_sb = w2p.tile([P, FT, C], bf16)
            w2r = w2d.rearrange("(t p) o -> p t o", p=P)
            for kt in range(FT):
                nc.gpsimd.dma_start(out=w2_sb[:, kt, :], in_=w2r[:, kt, :])
            gt_sb = gtp.tile([P, FT, RPC], bf16)
            for ft in range(FT):
                ps = ps_f1.tile([P, RPC], f32, tag="psf1")
                for kt in range(CT):
                    nc.tensor.matmul(
                        ps, lhsT=w1_sb[:, kt, ft * P:(ft + 1) * P],
                        rhs=ln2t[:, kt, :], start=(kt == 0), stop=(kt == CT - 1))
                nc.scalar.activation(
                    out=gt_sb[:, ft, :], in_=ps,
                    func=mybir.ActivationFunctionType.Gelu,
                    bias=b1_sb[:, ft:ft + 1], scale=1.0)
            for qt in range(QT):
                ps = ps_f2.tile([P, C], f32, tag="psf2")
                q0 = qt * P
                for c0, c1 in ((0, 512), (512, 768)):
                    nc.tensor.matmul(  # bias row -> psum (start of group)
                        ps[:, c0:c1], lhsT=ones1[0:1, :],
                        rhs=b2row[0:1, c0:c1], start=True, stop=False)
                    for kt in range(FT):
                        nc.tensor.matmul(
                            ps[:, c0:c1], lhsT=gt_sb[:, kt, q0:q0 + P],
                            rhs=w2_sb[:, kt, c0:c1],
                            start=False, stop=(kt == FT - 1))
                ot = outp.tile([P, C], f32, tag="ot")
                nc.vector.tensor_tensor(
                    out=ot, in0=ps, in1=x1_sb[:, qt, :], op=mybir.AluOpType.add)
                nc.sync.dma_start(out=outd[q0:q0 + P, :], in_=ot)

    nc.compile()
    return nc


def prep_inputs(x, attn_mask, ln1_g, ln1_b, w_qkv, b_qkv, w_proj, b_proj, ls1,
                ln2_g, ln2_b, w1, b1, w2, b2, ls2):
    """Host-side folding + per-core slicing. Returns list of 8 in_maps."""
    f = np.float32
    x = np.asarray(x, f)
    wqkv_f = (np.asarray(ln1_g, f)[:, None] * np.asarray(w_qkv, f)).astype(BF16)
    bqkv_f = (np.asarray(b_qkv, f)
              + np.asarray(ln1_b, f) @ np.asarray(w_qkv, f)).astype(f)
    wproj_f = (np.asarray(w_proj, f) * np.asarray(ls1, f)[None, :]) \
        .reshape(H, D, C).transpose(1, 0, 2).astype(BF16).copy()
    bproj_f = (np.asarray(b_proj, f) * np.asarray(ls1, f)).astype(f)
    w1_f = (np.asarray(ln2_g, f)[:, None] * np.asarray(w1, f)).astype(BF16)
    b1_f = (np.asarray(b1, f) + np.asarray(ln2_b, f) @ np.asarray(w1, f)).astype(f)
    w2_f = (np.asarray(w2, f) * np.asarray(ls2, f)[None, :]).astype(BF16)
    b2_f = (np.asarray(b2, f) * np.asarray(ls2, f)).astype(f)
    allowedT = (~np.asarray(attn_mask)).T.astype(BF16)  # [key, q] 1/0

    in_maps = []
    for c in range(NCORES):
        b, r = c // GPB, c % GPB
        xb_c = np.roll(x[b], -RPC * r, axis=0).copy()
        maskt_c = np.roll(allowedT[:, RPC * r:RPC * (r + 1)], -RPC * r,
                          axis=0).copy()
        in_maps.append({
            "xb": xb_c, "maskt": maskt_c, "wqkv": wqkv_f, "bqkv": bqkv_f,
            "wproj": wproj_f, "bproj": bproj_f, "w1d": w1_f, "b1d": b1_f,
            "w2d": w2_f, "b2d": b2_f,
        })
    return in_maps


_NC_CACHE = None


def kernel(**inputs):
    global _NC_CACHE
    if _NC_CACHE is None:
        _NC_CACHE = build_nc()
    nc = _NC_CACHE
    in_maps = prep_inputs(**inputs)
    res = run_bass_kernel_spmd(nc, in_maps, core_ids=list(range(NCORES)))
    kernel.last_result = res
    out = np.empty((B, N, C), np.float32)
    for c in range(NCORES):
        b, r = c // GPB, c % GPB
        out[b, RPC * r:RPC * (r + 1)] = res.results[c]["out"]
    return out


# revision 34
# speedup vs baseline: 1.2349x; 1.0159x over previous
"""Fused transformer block (LN1 -> MHA -> LN2 -> MLP, layerscale residuals)
for 8 Trainium2 NeuronCores.

Sharding: core c in 0..7 owns batch b = c//4 and query-row quarter r = c%4
(512 rows of 2048).  Each core recomputes K/V for its whole batch (no
cross-core communication), computes attention + proj + MLP for its own 512
rows, and writes a [512, 768] fp32 output slice.  Host assembles the full
[2, 2048, 768] output.

Dataflow is kept transposed where it kills transposes:
  LN1^T [C, rows] -> Q^T/K^T [cols, rows] (via w_qkv as lhsT) and V row-major
  [rows, cols] (via LN1^T as lhsT).  Scores are computed as S^T [keys, q]
  (lhsT = K^T head slice, rhs = Q^T head slice), exp'd on ACT with the 1/8
  scale, masked multiplicatively (exact: masked -> 0), and fed straight into
  AV as the moving operand with a ones-augmented V as lhsT so the softmax
  denominator falls out of the matmul's 65th output partition.  o^T [d, q]
  then feeds proj as lhsT; fc1 produces g^T [4C, q] which feeds fc2 as lhsT.

Host-side folding: LN gains/biases are absorbed into w_qkv/w1 + biases,
layerscales into w_proj/w2 + biases, so the kernel only ever normalizes
(x - mu) * rsigma.
"""

from contextlib import ExitStack

import numpy as np
import ml_dtypes

import concourse.bacc as bacc
import concourse.bass as bass
import concourse.tile as tile
from concourse import mybir
from concourse.bass_utils import run_bass_kernel_spmd
from concourse.masks import make_identity

BF16 = ml_dtypes.bfloat16

B, N, C, H, D = 2, 2048, 768, 12, 64
F = 4 * C            # 3072
P = 128
NCORES = 8
GPB = NCORES // B    # cores (row-quarters) per batch = 4
RPC = N // GPB       # rows per core = 512
KT = N // P          # key tiles = 16
CT = C // P          # channel tiles = 6
FT = F // P          # mlp hidden tiles = 24
QT = RPC // P        # own-row tiles = 4
EPS = 1e-5

f32 = mybir.dt.float32
bf16 = mybir.dt.bfloat16


def _ln_stats(nc, pool, xt, eps_tile, rows=P):
    """Return (mean, rstd) [rows,1] f32 for xt [rows, C] f32."""
    xr = xt.rearrange("p (s g) -> p s g", g=256)
    stats = pool.tile([P, 3, nc.vector.BN_STATS_DIM], f32, tag="ln_stats")
    for s in range(3):
        nc.vector.bn_stats(out=stats[:rows, s, :], in_=xr[:rows, s, :])
    mv = pool.tile([P, nc.vector.BN_AGGR_DIM], f32, tag="ln_mv")
    nc.vector.bn_aggr(out=mv[:rows], in_=stats[:rows])
    mean = mv[:rows, 0:1]
    rstd = pool.tile([P, 1], f32, tag="ln_rstd")
    # rstd <- 1/sqrt(var + eps)
    nc.scalar.activation(
        out=rstd[:rows], in_=mv[:rows, 1:2],
        func=mybir.ActivationFunctionType.Sqrt,
        bias=eps_tile[:rows], scale=1.0,
    )
    nc.vector.reciprocal(out=rstd[:rows], in_=rstd[:rows])
    return mean, rstd


def _ln_normalize_transpose(nc, pool, ps_tp, ident, xt, mean, rstd, lnt, col0,
                            out_pairs=CT // 2):
    """tbf = (xt - mean) * rstd (gpsimd, ->bf16), PE-transpose 128x128 blocks,
    drain pairs of blocks into lnt[:, ct, col0:col0+128]."""
    tbf = pool.tile([P, C], bf16, tag="tbf")
    nc.vector.tensor_scalar(
        out=tbf, in0=xt, scalar1=mean, scalar2=rstd,
        op0=mybir.AluOpType.subtract, op1=mybir.AluOpType.mult)
    for cp in range(out_pairs):
        tp = ps_tp.tile([P, 2, P], bf16, tag="tp")
        for j in range(2):
            ct = cp * 2 + j
            nc.tensor.transpose(tp[:, j, :], tbf[:, ct * P:(ct + 1) * P], ident)
        nc.vector.tensor_copy(out=lnt[:, cp * 2:cp * 2 + 2, col0:col0 + P],
                              in_=tp)


def build_nc():
    nc = bacc.Bacc(None, target_bir_lowering=False)

    xb = nc.dram_tensor("xb", [N, C], f32, kind="ExternalInput")
    maskt = nc.dram_tensor("maskt", [N, RPC], bf16, kind="ExternalInput")
    wqkv = nc.dram_tensor("wqkv", [C, 3 * C], bf16, kind="ExternalInput")
    bqkv = nc.dram_tensor("bqkv", [3 * C], f32, kind="ExternalInput")
    wproj = nc.dram_tensor("wproj", [D, H, C], bf16, kind="ExternalInput")
    bproj = nc.dram_tensor("bproj", [C], f32, kind="ExternalInput")
    w1d = nc.dram_tensor("w1d", [C, F], bf16, kind="ExternalInput")
    b1d = nc.dram_tensor("b1d", [F], f32, kind="ExternalInput")
    w2d = nc.dram_tensor("w2d", [F, C], bf16, kind="ExternalInput")
    b2d = nc.dram_tensor("b2d", [C], f32, kind="ExternalInput")
    outd = nc.dram_tensor("out", [RPC, C], f32, kind="ExternalOutput")

    def bcast_ap(vec, n):
        # [n] dram vector -> [P, n] partition-broadcast AP
        if not isinstance(vec, bass.AP):
            vec = vec[:]
        return bass.AP(tensor=vec.tensor, offset=vec.offset,
                       ap=[[0, P], *vec.ap])

    with tile.TileContext(nc) as tc, ExitStack() as root:
        consts = root.enter_context(tc.tile_pool(name="consts", bufs=1))
        ident = consts.tile([P, P], bf16)
        make_identity(nc, ident)
        eps_tile = consts.tile([P, 1], f32)
        nc.vector.memset(eps_tile, EPS)
        bqkv_sb = consts.tile([P, 18], f32)
        nc.sync.dma_start(out=bqkv_sb, in_=bqkv.rearrange("(t p) -> p t", p=P))
        b1_sb = consts.tile([P, FT], f32)
        nc.sync.dma_start(out=b1_sb, in_=b1d.rearrange("(t p) -> p t", p=P))
        ones1 = consts.tile([1, P], f32)
        nc.vector.memset(ones1, 1.0)
        bprow = consts.tile([1, C], f32)
        nc.gpsimd.dma_start(out=bprow, in_=bcast_ap(bproj, C)[0:1, :])
        b2row = consts.tile([1, C], f32)
        nc.gpsimd.dma_start(out=b2row, in_=bcast_ap(b2d, C)[0:1, :])
        bv_bc = consts.tile([P, C], f32)
        nc.gpsimd.dma_start(out=bv_bc, in_=bcast_ap(bqkv[2 * C:3 * C], C))

        # Long-lived pools, created in reverse order of release (LIFO alloc):
        x1p = root.enter_context(tc.tile_pool(name="x1_p", bufs=1))
        ln2p = root.enter_context(tc.tile_pool(name="ln2t_p", bufs=1))
        x1_sb = x1p.tile([P, QT, C], f32)
        ln2t = ln2p.tile([P, CT, RPC], bf16)
        sBC = root.enter_context(ExitStack())
        otp = sBC.enter_context(tc.tile_pool(name="ot_p", bufs=1))
        ot_sb = otp.tile([D, H, RPC], bf16)

        # ---------------- Phase A: LN1 + Q^T/V (interleaved) --------------
        # Pools living through attention (freed before MLP):
        sAB = root.enter_context(ExitStack())
        qk_pool = sAB.enter_context(tc.tile_pool(name="qk_sb", bufs=1))
        vaug_pool = sAB.enter_context(tc.tile_pool(name="vaug_sb", bufs=1))
        wp = sAB.enter_context(tc.tile_pool(name="wqkv_p", bufs=1))
        lp = sAB.enter_context(tc.tile_pool(name="ln1t_p", bufs=1))
        ktp = sAB.enter_context(tc.tile_pool(name="kt_p", bufs=2))
        qt_sb = qk_pool.tile([P, CT, RPC], bf16)
        maskt_sb = qk_pool.tile([P, KT, RPC], bf16)
        vaug = vaug_pool.tile([P, KT, H, D + 1], bf16)
        nc.vector.memset(vaug[:, :, :, D:D + 1], 1.0)
        wqkv_sb = wp.tile([P, CT, 3 * C], bf16)
        wqkvr = wqkv.rearrange("(t p) o -> p t o", p=P)
        wqkv_dmas = []
        for kt in range(CT):  # v columns first: V matmuls start earliest
            wqkv_dmas.append(
                nc.gpsimd.dma_start(out=wqkv_sb[:, kt, 2 * C:3 * C],
                                    in_=wqkvr[:, kt, 2 * C:3 * C]))
        for kt in range(CT):
            wqkv_dmas.append(
                nc.gpsimd.dma_start(out=wqkv_sb[:, kt, 0:2 * C],
                                    in_=wqkvr[:, kt, 0:2 * C]))
        ln1t = lp.tile([P, CT, N], bf16)

        def kt_chunk(dst, mt, nt, ps_pool, tag):
            """K^T chunk: cols (CT+mt)*128.., rows nt*512.. -> dst[:, chunk]."""
            ps = ps_pool.tile([P, 512], f32, tag=tag, name="ps_ktc")
            for kt in range(CT):
                nc.tensor.matmul(
                    ps, lhsT=wqkv_sb[:, kt, (CT + mt) * P:(CT + mt + 1) * P],
                    rhs=ln1t[:, kt, nt * 512:(nt + 1) * 512],
                    start=(kt == 0), stop=(kt == CT - 1))
            nc.vector.tensor_scalar(
                out=dst[:, nt * 512:(nt + 1) * 512], in0=ps,
                scalar1=bqkv_sb[:, CT + mt:CT + mt + 1], scalar2=None,
                op0=mybir.AluOpType.add)

        kt0_sb = ktp.tile([P, N], bf16, tag="kt", name="kt0_sb")
        with ExitStack() as sA:
            lnw = sA.enter_context(tc.tile_pool(name="ln1_work", bufs=3))
            ps_tp = sA.enter_context(tc.tile_pool(name="ps_tp", bufs=2, space="PSUM"))
            ps_v = sA.enter_context(tc.tile_pool(name="ps_v", bufs=2, space="PSUM"))
            ps_qt = sA.enter_context(tc.tile_pool(name="ps_qt", bufs=2, space="PSUM"))
            for rt in range(KT):
                xt = lnw.tile([P, C], f32, tag="xt")
                xt_dma = nc.sync.dma_start(out=xt, in_=xb[rt * P:(rt + 1) * P, :])
                if rt == 0:
                    # keep the first activations load ahead of the bulk
                    # weight traffic on HBM
                    for wdma in wqkv_dmas:
                        tile.add_dep_helper(wdma.ins, xt_dma.ins,
                                            reason="x before weights")
                mean, rstd = _ln_stats(nc, lnw, xt, eps_tile)
                _ln_normalize_transpose(nc, lnw, ps_tp, ident, xt, mean, rstd,
                                        ln1t, rt * P)
                # V for this key tile
                ps = ps_v.tile([P, C], f32, tag="psv")
                for c0, c1 in ((0, 512), (512, 768)):
                    for kt in range(CT):
                        nc.tensor.matmul(
                            ps[:, c0:c1],
                            lhsT=ln1t[:, kt, rt * P:(rt + 1) * P],
                            rhs=wqkv_sb[:, kt, 2 * C + c0:2 * C + c1],
                            start=(kt == 0), stop=(kt == CT - 1))
                nc.vector.tensor_tensor(
                    out=vaug[:, rt, :, 0:D],
                    in0=ps.rearrange("p (h d) -> p h d", d=D),
                    in1=bv_bc.rearrange("p (h d) -> p h d", d=D),
                    op=mybir.AluOpType.add)
                if rt == 3:
                    # own rows done -> Q^T for all heads
                    for mt in range(CT):
                        ps = ps_qt.tile([P, RPC], f32, tag="psqt")
                        for kt in range(CT):
                            nc.tensor.matmul(
                                ps, lhsT=wqkv_sb[:, kt, mt * P:(mt + 1) * P],
                                rhs=ln1t[:, kt, 0:RPC],
                                start=(kt == 0), stop=(kt == CT - 1))
                        nc.vector.tensor_scalar(
                            out=qt_sb[:, mt, :], in0=ps,
                            scalar1=bqkv_sb[:, mt:mt + 1], scalar2=None,
                            op0=mybir.AluOpType.add)
                if rt % 4 == 3:
                    # K^T head-pair 0, row chunk rt//4 (rows just completed)
                    kt_chunk(kt0_sb, 0, rt // 4, ps_qt, "psqt")

        # ------------- Phase B: fused K^T generation + attention ----------
        # Per mt (head pair): scores(2mt), scores(2mt+1), K^T(mt+1), AV pair.
        # The K^T matmuls keep PE busy while ACT runs the exps.
        nc.gpsimd.dma_start(out=maskt_sb,  # late: don't contend with xt loads
                            in_=maskt.rearrange("(t p) q -> p t q", p=P))
        with ExitStack() as sB:
            ptp = sB.enter_context(tc.tile_pool(name="pt_p", bufs=3))
            recp = sB.enter_context(tc.tile_pool(name="rec_p", bufs=2))
            dramp = sB.enter_context(tc.tile_pool(name="den_dram", bufs=2,
                                                  space="DRAM"))
            ps_kt = sB.enter_context(tc.tile_pool(name="ps_kt", bufs=2, space="PSUM"))
            ps_s = sB.enter_context(tc.tile_pool(name="ps_s", bufs=2, space="PSUM"))
            ps_av = sB.enter_context(tc.tile_pool(name="ps_av", bufs=2, space="PSUM"))
            kt_tiles = {0: kt0_sb}

            def scores_group(h, g, pt):
                po = D * (h % 2)
                mt = h // 2
                sps = ps_s.tile([P, 2, RPC], f32, tag="sps", name="sps")
                for j in range(2):
                    k0 = (g * 2 + j) * P
                    nc.tensor.matmul(
                        sps[:, j, :],
                        lhsT=kt_tiles[mt][po:po + D, k0:k0 + P],
                        rhs=qt_sb[po:po + D, mt, :],
                        start=True, stop=True)
                nc.scalar.activation(
                    out=pt[:, g * 2:g * 2 + 2, :], in_=sps,
                    func=mybir.ActivationFunctionType.Exp,
                    scale=float(D) ** -0.5)
                nc.vector.tensor_tensor(
                    out=pt[:, g * 2:g * 2 + 2, :],
                    in0=pt[:, g * 2:g * 2 + 2, :],
                    in1=maskt_sb[:, g * 2:g * 2 + 2, :],
                    op=mybir.AluOpType.mult)

            def av_part(h, avps, pt, g):
                for j in range(2):
                    kt = g * 2 + j
                    nc.tensor.matmul(
                        avps, lhsT=vaug[:, kt, h, :], rhs=pt[:, kt, :],
                        start=(kt == 0), stop=(kt == KT - 1))

            def finish_head(h, avps):
                # denominator -> 1/denominator -> broadcast over 64 partitions.
                # The exact vector.reciprocal is per-lane serial, so bounce the
                # [1,512] row through DRAM reshaped to [128,4] to use all
                # lanes, then bounce back with a 0-stride partition broadcast
                # (legal for DRAM sources only).
                den = recp.tile([D + 1, RPC], f32, tag="den", name="den")
                nc.vector.tensor_copy(out=den[D:D + 1, :], in_=avps[D:D + 1, :])
                dscr = dramp.tile([1, RPC], f32, name="dscr")
                nc.gpsimd.dma_start(out=dscr[:, :], in_=den[D:D + 1, :])
                rsm = recp.tile([P, RPC // P], f32, tag="rsm", name="rsm")
                nc.gpsimd.dma_start(
                    out=rsm,
                    in_=bass.AP(tensor=dscr.tensor, offset=dscr.offset,
                                ap=[[RPC // P, P], [1, RPC // P]]))
                nc.vector.reciprocal(out=rsm, in_=rsm)
                dscr2 = dramp.tile([1, RPC], f32, name="dscr2")
                nc.gpsimd.dma_start(
                    out=bass.AP(tensor=dscr2.tensor, offset=dscr2.offset,
                                ap=[[RPC // P, P], [1, RPC // P]]),
                    in_=rsm)
                den_full = recp.tile([D, RPC], f32, tag="den_bc", name="den_full")
                nc.gpsimd.dma_start(
                    out=den_full,
                    in_=bass.AP(tensor=dscr2.tensor, offset=dscr2.offset,
                                ap=[[0, D], [1, RPC]]))
                nc.vector.tensor_tensor(
                    out=ot_sb[:, h, :], in0=avps[0:D, :], in1=den_full,
                    op=mybir.AluOpType.mult)

            def dummy_warm(n):
                # dependency-free transposes that keep the PE HAM counter
                # busy across stalls (writes go to a throwaway psum tile)
                kw = ps_kt.tile([P, 2, P], bf16, tag="psktc", name="kw")
                for i in range(n):
                    nc.tensor.transpose(kw[:, i % 2, :], ident, ident)

            # One-head-delayed pipeline: scores(h) weave with AV(h-1) and the
            # next head pair's K^T chunks (2 per head), so the in-order PE
            # FIFO never waits on ACT.
            pt_prev = None
            for h in range(H):
                pt = ptp.tile([P, KT, RPC], bf16, tag="pt", name="pt")
                avps = None
                p_next = h // 2 + 1
                for g in range(KT // 2):
                    scores_group(h, g, pt)
                    if g in (3, 7):
                        if p_next < CT:
                            if p_next not in kt_tiles:
                                kt_tiles[p_next] = ktp.tile(
                                    [P, N], bf16, tag="kt", name="kt_sb_n")
                            nt = (0 if g == 3 else 1) + 2 * (h % 2)
                            kt_chunk(kt_tiles[p_next], p_next, nt,
                                     ps_kt, "psktc")
                        else:
                            dummy_warm(4)
                    elif g in (1, 5) and p_next >= CT:
                        dummy_warm(4)
                    if pt_prev is not None:
                        if avps is None:
                            avps = ps_av.tile([D + 1, RPC], f32, tag="avps",
                                              name="avps")
                        av_part(h - 1, avps, pt_prev, g)
                if pt_prev is not None:
                    finish_head(h - 1, avps)
                pt_prev = pt
            # epilogue: AV + normalize for the last head
            avps = ps_av.tile([D + 1, RPC], f32, tag="avps", name="avps")
            for g in range(KT // 2):
                dummy_warm(3)
                av_part(H - 1, avps, pt_prev, g)
            finish_head(H - 1, avps)
        sAB.close()  # free qt/kt/vaug/wqkv before the MLP weights arrive

        # ---------------- Phase C: proj + residual + LN2 ----------------
        with ExitStack() as sC:
            wpp = sC.enter_context(tc.tile_pool(name="wproj_p", bufs=1))
            xqp = sC.enter_context(tc.tile_pool(name="xq_p", bufs=4))
            ln2w = sC.enter_context(tc.tile_pool(name="ln2_work", bufs=4))
            ps_pj = sC.enter_context(tc.tile_pool(name="ps_pj", bufs=3, space="PSUM"))
            ps_t2 = sC.enter_context(tc.tile_pool(name="ps_t2", bufs=2, space="PSUM"))
            wproj_sb = wpp.tile([D, H, C], bf16)
            nc.gpsimd.dma_start(out=wproj_sb, in_=wproj[:, :, :])
            # pass 1: all proj matmul groups back-to-back (PE stays dense),
            # residual adds drain them as they complete
            t2s = []
            for qt in range(QT):
                ps = ps_pj.tile([P, C], f32, tag="pspj")
                q0 = qt * P
                for c0, c1 in ((0, 512), (512, 768)):
                    nc.tensor.matmul(  # bias row -> psum (start of group)
                        ps[:, c0:c1], lhsT=ones1[0:1, :],
                        rhs=bprow[0:1, c0:c1], start=True, stop=False)
                    for h in range(H):
                        nc.tensor.matmul(
                            ps[:, c0:c1], lhsT=ot_sb[:, h, q0:q0 + P],
                            rhs=wproj_sb[:, h, c0:c1],
                            start=False, stop=(h == H - 1))
                xq = xqp.tile([P, C], f32, tag="xq")
                nc.sync.dma_start(out=xq, in_=xb[q0:q0 + P, :])
                nc.vector.tensor_tensor(
                    out=x1_sb[:, qt, :], in0=ps, in1=xq, op=mybir.AluOpType.add)
                mean2, rstd2 = _ln_stats(nc, ln2w, x1_sb[:, qt, :], eps_tile)
                t2 = ln2w.tile([P, C], bf16, tag="t2")
                nc.vector.tensor_scalar(
                    out=t2, in0=x1_sb[:, qt, :], scalar1=mean2, scalar2=rstd2,
                    op0=mybir.AluOpType.subtract, op1=mybir.AluOpType.mult)
                t2s.append(t2)
            # pass 2: all LN2 transposes after the proj groups in the PE FIFO
            for qt in range(QT):
                q0 = qt * P
                for cp in range(CT // 2):
                    tp2 = ps_t2.tile([P, 2, P], bf16, tag="tp2")
                    for j in range(2):
                        ct = cp * 2 + j
                        nc.tensor.transpose(
                            tp2[:, j, :], t2s[qt][:, ct * P:(ct + 1) * P], ident)
                    nc.vector.tensor_copy(
                        out=ln2t[:, cp * 2:cp * 2 + 2, q0:q0 + P], in_=tp2)
        sBC.close()  # free ot_sb

        # ---------------- Phase D: MLP ----------------
        with ExitStack() as sD:
            w1p = sD.enter_context(tc.tile_pool(name="w1_p", bufs=1))
            w2p = sD.enter_context(tc.tile_pool(name="w2_p", bufs=1))
            gtp = sD.enter_context(tc.tile_pool(name="gt_p", bufs=1))
            outp = sD.enter_context(tc.tile_pool(name="out_p", bufs=2))
            ps_f1 = sD.enter_context(tc.tile_pool(name="ps_f1", bufs=3, space="PSUM"))
            ps_f2 = sD.enter_context(tc.tile_pool(name="ps_f2", bufs=2, space="PSUM"))
            w1_sb = w1p.tile([P, CT, F], bf16)
            w1r = w1d.rearrange("(t p) f -> p t f", p=P)
            for kt in range(CT):  # chunked so fc1 can start on the first k-tile
                nc.gpsimd.dma_start(out=w1_sb[:, kt, :], in_=w1r[:, kt, :])
            w2_sb = w2p.tile([P, FT, C], bf16)
            w2r = w2d.rearrange("(t p) o -> p t o", p=P)
            for kt in range(FT):
                nc.gpsimd.dma_start(out=w2_sb[:, kt, :], in_=w2r[:, kt, :])
            gt_sb = gtp.tile([P, FT, RPC], bf16)
            for ft in range(FT):
                ps = ps_f1.tile([P, RPC], f32, tag="psf1")
                for kt in range(CT):
                    nc.tensor.matmul(
                        ps, lhsT=w1_sb[:, kt, ft * P:(ft + 1) * P],
                        rhs=ln2t[:, kt, :], start=(kt == 0), stop=(kt == CT - 1))
                nc.scalar.activation(
                    out=gt_sb[:, ft, :], in_=ps,
                    func=mybir.ActivationFunctionType.Gelu,
                    bias=b1_sb[:, ft:ft + 1], scale=1.0)
            for qt in range(QT):
                ps = ps_f2.tile([P, C], f32, tag="psf2")
                q0 = qt * P
                for c0, c1 in ((0, 512), (512, 768)):
                    nc.tensor.matmul(  # bias row -> psum (start of group)
                        ps[:, c0:c1], lhsT=ones1[0:1, :],
                        rhs=b2row[0:1, c0:c1], start=True, stop=False)
                    for kt in range(FT):
                        nc.tensor.matmul(
                            ps[:, c0:c1], lhsT=gt_sb[:, kt, q0:q0 + P],
                            rhs=w2_sb[:, kt, c0:c1],
                            start=False, stop=(kt == FT - 1))
                ot = outp.tile([P, C], f32, tag="ot")
                nc.vector.tensor_tensor(
                    out=ot, in0=ps, in1=x1_sb[:, qt, :], op=mybir.AluOpType.add)
                nc.sync.dma_start(out=outd[q0:q0 + P, :], in_=ot)

    nc.compile()
    return nc


def prep_inputs(x, attn_mask, ln1_g, ln1_b, w_qkv, b_qkv, w_proj, b_proj, ls1,
                ln2_g, ln2_b, w1, b1, w2, b2, ls2):
    """Host-side folding + per-core slicing. Returns list of 8 in_maps."""
    f = np.float32
    x = np.asarray(x, f)
    wqkv_f = (np.asarray(ln1_g, f)[:, None] * np.asarray(w_qkv, f)).astype(BF16)
    bqkv_f = (np.asarray(b_qkv, f)
              + np.asarray(ln1_b, f) @ np.asarray(w_qkv, f)).astype(f)
    wproj_f = (np.asarray(w_proj, f) * np.asarray(ls1, f)[None, :]) \
        .reshape(H, D, C).transpose(1, 0, 2).astype(BF16).copy()
    bproj_f = (np.asarray(b_proj, f) * np.asarray(ls1, f)).astype(f)
    w1_f = (np.asarray(ln2_g, f)[:, None] * np.asarray(w1, f)).astype(BF16)
    b1_f = (np.asarray(b1, f) + np.asarray(ln2_b, f) @ np.asarray(w1, f)).astype(f)
    w2_f = (np.asarray(w2, f) * np.asarray(ls2, f)[None, :]).astype(BF16)
    b2_f = (np.asarray(b2, f) * np.asarray(ls2, f)).astype(f)
    allowedT = (~np.asarray(attn_mask)).T.astype(BF16)  # [key, q] 1/0

    in_maps = []
    for c in range(NCORES):
        b, r = c // GPB, c % GPB
        xb_c = np.roll(x[b], -RPC * r, axis=0).copy()
        maskt_c = np.roll(allowedT[:, RPC * r:RPC * (r + 1)], -RPC * r,
                          axis=0).copy()
        in_maps.append({
            "xb": xb_c, "maskt": maskt_c, "wqkv": wqkv_f, "bqkv": bqkv_f,
            "wproj": wproj_f, "bproj": bproj_f, "w1d": w1_f, "b1d": b1_f,
            "w2d": w2_f, "b2d": b2_f,
        })
    return in_maps


_NC_CACHE = None


def kernel(**inputs):
    global _NC_CACHE
    if _NC_CACHE is None:
        _NC_CACHE = build_nc()
    nc = _NC_CACHE
    in_maps = prep_inputs(**inputs)
    res = run_bass_kernel_spmd(nc, in_maps, core_ids=list(range(NCORES)))
    kernel.last_result = res
    out = np.empty((B, N, C), np.float32)
    for c in range(NCORES):
        b, r = c // GPB, c % GPB
        out[b, RPC * r:RPC * (r + 1)] = res.results[c]["out"]
    return out
